# revision 6
# baseline (speedup 1.0000x reference)
"""Trainium2 Bass kernel for single-head attention with row-major K-reshape.

Reference computation (per batch b):
    Q = x @ W_Q.T ; K = x @ W_K.T ; V = x @ W_V.T          # [S, D]
    K_r = K.reshape(D, S)          # row-major reshape, NOT a transpose
    scores = Q @ K_r / D
    out = softmax(scores, -1) @ V

Shapes: B=4, S=2048, D=1024, f32.

Sharding: 8 cores = (batch b in 0..3) x (query half h in 0..1). Each core
computes out[b, h*1024:(h+1)*1024, :] from full x[b] (K/V need the whole
sequence) plus its query block. No collectives.

Key layout identity: with S == 2*D the row-major reshape gives
    K_r[m, j]      = K[2m, j]        for j <  D
    K_r[m, D + j]  = K[2m+1, j]      for j <  D
so K_r is produced DIRECTLY by the K projection using stride-2 column
slices of x^T as the stationary operand:
    K_r[m, j]     = sum_d x^T[d, 2m]   * W_K^T[d, j]
    K_r[m, D + j] = sum_d x^T[d, 2m+1] * W_K^T[d, j]

Dataflow per core (TensorE matmul computes out[M,N] = lhsT[K,M].T @ rhs[K,N],
contraction over the partition dim):
    xT[d, s], xqT[d, i], w*T[d, c]:  f32->bf16 cast DMAs (SWDGE, DRAM->DRAM
        scratch) followed by xbar transpose-DMA loads into SBUF.
    QT[m, i]   = lhsT=wqT[:, m-slice],        rhs=xqT            (proj)
    KR[m, j]   = lhsT=xT[:, stride-2 slice],  rhs=wkT            (proj)
    V[s, c]    = lhsT=xT[:, s-slice],         rhs=wvT            (proj)
    ST[j, i]   = lhsT=KR[:, j-slice],         rhs=QT             (scores^T)
    ET[j, i]   = exp(ST / D)                  (ACT, psum->sbuf bf16)
    rsum[i, 1] = lhsT=ET[:, i-slice],         rhs=ones[128, 1]   (row sums)
    O[i, c]    = lhsT=ET[:, i-slice],         rhs=V              (out)
    out        = O * (1 / rsum)               (DVE per-partition scalar)

All matmul operands bf16 (1 cycle/row on PE), accumulation f32 in PSUM.
"""

from contextlib import ExitStack

import numpy as np

import concourse.bass as bass
import concourse.tile as tile
from concourse import bacc, mybir
from concourse.bass_utils import run_bass_kernel_spmd

F32 = mybir.dt.float32
BF16 = mybir.dt.bfloat16
P = 128


def build_attention(nc, S=2048, D=1024, QB=1024):
    """Emit the per-core attention program into `nc`. Requires S == 2*D."""
    assert S == 2 * D and S % P == 0 and D % P == 0 and QB % P == 0
    NST = S // P        # seq tiles (16)
    NDT = D // P        # d_model tiles (8)
    NQT = QB // P       # query tiles for this core (8)
    NC = min(512, D, QB)  # matmul free-dim chunk (one PSUM bank of f32)
    NCH_D = D // NC     # chunks over output channels (2)
    NCH_Q = QB // NC    # chunks over queries (2)
    EXP = mybir.ActivationFunctionType.Exp

    x_ap = nc.dram_tensor("x", [S, D], F32, kind="ExternalInput").ap()
    xq_ap = nc.dram_tensor("xq", [QB, D], F32, kind="ExternalInput").ap()
    w_aps = {
        w: nc.dram_tensor(w, [D, D], F32, kind="ExternalInput").ap()
        for w in ("wq", "wk", "wv")
    }
    out_ap = nc.dram_tensor("out", [QB, D], F32, kind="ExternalOutput").ap()

    CAST_ROWS = 512  # rows per f32->bf16 cast DMA chunk
    TR_ROWS = 512    # rows per xbar-transpose DMA chunk

    with tile.TileContext(nc) as tc, ExitStack() as ctx:
        const_pool = ctx.enter_context(tc.tile_pool(name="const", bufs=1))
        qt_pool = ctx.enter_context(tc.tile_pool(name="qt", bufs=1))
        kr_pool = ctx.enter_context(tc.tile_pool(name="kr", bufs=1))
        v_pool = ctx.enter_context(tc.tile_pool(name="v", bufs=1))
        dram = ctx.enter_context(tc.tile_pool(name="dram", bufs=1, space="DRAM"))
        psum_mm = ctx.enter_context(tc.tile_pool(name="psum_mm", bufs=6, space="PSUM"))
        psum_r = ctx.enter_context(tc.tile_pool(name="psum_r", bufs=2, space="PSUM"))

        ones = const_pool.tile([P, 1], BF16)
        nc.vector.memset(ones, 1.0)

        QT = [qt_pool.tile([P, QB], BF16, tag=f"QT{m}", name=f"QT{m}") for m in range(NDT)]
        KR = [kr_pool.tile([P, S], BF16, tag=f"KR{m}", name=f"KR{m}") for m in range(NDT)]
        V = [v_pool.tile([P, D], BF16, tag=f"V{s}", name=f"V{s}") for s in range(NST)]

        def cast_bf16(src_ap, nrows):
            # f32 DRAM -> bf16 DRAM scratch, chunked cast DMAs on SWDGE
            dst = dram.tile([nrows, D], BF16, name=f"bf_{src_ap.tensor.name}")
            for r0 in range(0, nrows, CAST_ROWS):
                r1 = min(r0 + CAST_ROWS, nrows)
                nc.gpsimd.dma_start(out=dst[r0:r1, :], in_=src_ap[r0:r1, :])
            return dst

        def load_transposed(src_bf, nrows, dstT):
            # bf16 DRAM [nrows, D] -> dstT: NDT bf16 SBUF tiles [P, nrows]
            for dt in range(NDT):
                for r0 in range(0, nrows, TR_ROWS):
                    r1 = min(r0 + TR_ROWS, nrows)
                    nc.sync.dma_start(
                        out=dstT[dt][:, r0:r1],
                        in_=src_bf[r0:r1, dt * P:(dt + 1) * P],
                        transpose=True,
                    )

        with tc.tile_pool(name="xt", bufs=1) as xt_pool, \
                tc.tile_pool(name="wt", bufs=2) as wt_pool:

            xq_bf = cast_bf16(xq_ap, QB)
            x_bf = cast_bf16(x_ap, S)
            xqT = [xt_pool.tile([P, QB], BF16, tag=f"xqT{d}", name=f"xqT{d}") for d in range(NDT)]
            load_transposed(xq_bf, QB, xqT)
            xT = [xt_pool.tile([P, S], BF16, tag=f"xT{d}", name=f"xT{d}") for d in range(NDT)]
            load_transposed(x_bf, S, xT)

            # stride-2 views of xT: [P, 2, S//2]; [:, half, m] = xT[:, 2m+half]
            xTe = [t.rearrange("p (m two) -> p two m", two=2) for t in xT]

            for wname in ("wq", "wk", "wv"):
                w_bf = cast_bf16(w_aps[wname], D)
                wT = [wt_pool.tile([P, D], BF16, tag=f"wT{d}", name=f"wT{d}") for d in range(NDT)]
                load_transposed(w_bf, D, wT)

                if wname == "wq":
                    # QT[mt][:, ich] = sum_dt wqT[dt][:, m-slice].T @ xqT[dt][:, ich]
                    for mt in range(NDT):
                        for ich in range(NCH_Q):
                            pm = psum_mm.tile([P, NC], F32, tag="pm")
                            for dt in range(NDT):
                                nc.tensor.matmul(
                                    pm[:],
                                    wT[dt][:, mt * P:(mt + 1) * P],
                                    xqT[dt][:, ich * NC:(ich + 1) * NC],
                                    start=(dt == 0), stop=(dt == NDT - 1),
                                )
                            nc.scalar.copy(QT[mt][:, ich * NC:(ich + 1) * NC], pm[:])
                elif wname == "wk":
                    # KR[mt][:, half*D + jch] = sum_dt xT[dt][:, 2m+half].T @ wkT[dt][:, jch]
                    for half in range(2):
                        for mt in range(NDT):
                            for jch in range(NCH_D):
                                pm = psum_mm.tile([P, NC], F32, tag="pm")
                                for dt in range(NDT):
                                    nc.tensor.matmul(
                                        pm[:],
                                        xTe[dt][:, half, mt * P:(mt + 1) * P],
                                        wT[dt][:, jch * NC:(jch + 1) * NC],
                                        start=(dt == 0), stop=(dt == NDT - 1),
                                    )
                                nc.scalar.copy(
                                    KR[mt][:, half * D + jch * NC: half * D + (jch + 1) * NC],
                                    pm[:],
                                )
                else:
                    # V[st][:, cch] = sum_dt xT[dt][:, s-slice].T @ wvT[dt][:, cch]
                    for st in range(NST):
                        for cch in range(NCH_D):
                            pm = psum_mm.tile([P, NC], F32, tag="pm")
                            for dt in range(NDT):
                                nc.tensor.matmul(
                                    pm[:],
                                    xT[dt][:, st * P:(st + 1) * P],
                                    wT[dt][:, cch * NC:(cch + 1) * NC],
                                    start=(dt == 0), stop=(dt == NDT - 1),
                                )
                            nc.vector.tensor_copy(V[st][:, cch * NC:(cch + 1) * NC], pm[:])

        with tc.tile_pool(name="et", bufs=1) as et_pool, \
                tc.tile_pool(name="ostage", bufs=3) as ostage, \
                tc.tile_pool(name="recip", bufs=1) as recip_pool:

            # scores^T and exp: ET[jt][:, ich] = exp(sum_mt KR.T @ QT / D)
            ET = [et_pool.tile([P, QB], BF16, tag=f"ET{j}", name=f"ET{j}") for j in range(NST)]
            for jt in range(NST):
                for ich in range(NCH_Q):
                    pm = psum_mm.tile([P, NC], F32, tag="pm")
                    for mt in range(NDT):
                        nc.tensor.matmul(
                            pm[:],
                            KR[mt][:, jt * P:(jt + 1) * P],
                            QT[mt][:, ich * NC:(ich + 1) * NC],
                            start=(mt == 0), stop=(mt == NDT - 1),
                        )
                    nc.scalar.activation(
                        ET[jt][:, ich * NC:(ich + 1) * NC], pm[:], EXP, scale=1.0 / D
                    )

            # softmax denominators, directly in [query-partition, 1] layout
            recips = []
            for it in range(NQT):
                pr = psum_r.tile([P, 1], F32, tag="pr")
                for jt in range(NST):
                    nc.tensor.matmul(
                        pr[:], ET[jt][:, it * P:(it + 1) * P], ones[:],
                        start=(jt == 0), stop=(jt == NST - 1),
                    )
                rc = recip_pool.tile([P, 1], F32, tag=f"rc{it}", name=f"rc{it}")
                nc.vector.reciprocal(rc[:], pr[:])
                recips.append(rc)

            # out[it][:, cch] = (sum_jt ET.T @ V) * recip[it]
            for it in range(NQT):
                for cch in range(NCH_D):
                    pm = psum_mm.tile([P, NC], F32, tag="pm")
                    for jt in range(NST):
                        nc.tensor.matmul(
                            pm[:],
                            ET[jt][:, it * P:(it + 1) * P],
                            V[jt][:, cch * NC:(cch + 1) * NC],
                            start=(jt == 0), stop=(jt == NST - 1),
                        )
                    ob = ostage.tile([P, NC], F32, tag="ob")
                    nc.vector.tensor_scalar_mul(ob[:], pm[:], recips[it][:])
                    nc.sync.dma_start(
                        out=out_ap[it * P:(it + 1) * P, cch * NC:(cch + 1) * NC],
                        in_=ob[:],
                    )
    return nc


_CACHE = {}


def _get_nc(S=2048, D=1024, QB=1024):
    key = (S, D, QB)
    if key not in _CACHE:
        nc = bacc.Bacc("TRN2", target_bir_lowering=False, debug=False, num_devices=8)
        build_attention(nc, S=S, D=D, QB=QB)
        nc.compile()
        _CACHE[key] = nc
    return _CACHE[key]


def _run(x, W_Q, W_K, W_V, **spmd_kwargs):
    B, S, D = x.shape  # (4, 2048, 1024)
    QB = S * B // 8    # queries per core (1024)
    x = np.ascontiguousarray(np.asarray(x, dtype=np.float32))
    ws = {
        "wq": np.ascontiguousarray(np.asarray(W_Q, dtype=np.float32)),
        "wk": np.ascontiguousarray(np.asarray(W_K, dtype=np.float32)),
        "wv": np.ascontiguousarray(np.asarray(W_V, dtype=np.float32)),
    }
    nc = _get_nc(S=S, D=D, QB=QB)
    in_maps = []
    for core in range(8):
        b, h = core // 2, core % 2
        in_maps.append({
            "x": x[b],
            "xq": np.ascontiguousarray(x[b, h * QB:(h + 1) * QB, :]),
            **ws,
        })
    res = run_bass_kernel_spmd(nc, in_maps, list(range(8)), **spmd_kwargs)
    out = np.empty((B, S, D), dtype=np.float32)
    for core in range(8):
        b, h = core // 2, core % 2
        out[b, h * QB:(h + 1) * QB, :] = res.results[core]["out"]
    return out, res


def kernel(x, W_Q, W_K, W_V):
    return _run(x, W_Q, W_K, W_V)[0]


# revision 10
# speedup vs baseline: 1.0527x; 1.0527x over previous
"""Trainium2 Bass kernel for single-head attention with row-major K-reshape.

Reference computation (per batch b):
    Q = x @ W_Q.T ; K = x @ W_K.T ; V = x @ W_V.T          # [S, D]
    K_r = K.reshape(D, S)          # row-major reshape, NOT a transpose
    scores = Q @ K_r / D
    out = softmax(scores, -1) @ V

Shapes: B=4, S=2048, D=1024, f32.

Sharding: 8 cores = (batch b in 0..3) x (query half h in 0..1). Each core
computes out[b, h*1024:(h+1)*1024, :] from full x[b] (K/V need the whole
sequence) plus its query block. No collectives.

Key layout identity: with S == 2*D the row-major reshape gives
    K_r[m, j]      = K[2m, j]        for j <  D
    K_r[m, D + j]  = K[2m+1, j]      for j <  D
so K_r is produced DIRECTLY by the K projection using stride-2 column
slices of x^T as the stationary operand:
    K_r[m, j]     = sum_d x^T[d, 2m]   * W_K^T[d, j]
    K_r[m, D + j] = sum_d x^T[d, 2m+1] * W_K^T[d, j]

Dataflow per core (TensorE matmul computes out[M,N] = lhsT[K,M].T @ rhs[K,N],
contraction over the partition dim):
    xT[d, s], xqT[d, i]: PE 128x128 transposes of the f32 natural tiles,
        bf16 cast on the PSUM->SBUF copy (DVE).  w*T[d, c]: f32->bf16 cast
        DMAs (SWDGE, DRAM->DRAM scratch) + xbar transpose-DMA loads.
    QT[m, i]   = lhsT=wqT[:, m-slice],        rhs=xqT            (proj)
    KR[m, j]   = lhsT=xT[:, stride-2 slice],  rhs=wkT            (proj)
    V[s, c]    = lhsT=xT[:, s-slice],         rhs=wvT            (proj)
    ST[j, i]   = lhsT=KR[:, j-slice],         rhs=QT             (scores^T)
    ET[j, i]   = exp(ST / D)                  (ACT, psum->sbuf bf16)
    rsum[i, 1] = lhsT=ET[:, i-slice],         rhs=ones[128, 1]   (row sums)
    O[i, c]    = lhsT=ET[:, i-slice],         rhs=V              (out)
    out        = O * (1 / rsum)               (DVE per-partition scalar)

All matmul operands bf16 (1 cycle/row on PE), accumulation f32 in PSUM.
"""

from contextlib import ExitStack

import numpy as np

import concourse.bass as bass
import concourse.tile as tile
from concourse import bacc, mybir
from concourse.bass_utils import run_bass_kernel_spmd
from concourse.masks import make_identity

F32 = mybir.dt.float32
BF16 = mybir.dt.bfloat16
P = 128


def build_attention(nc, S=2048, D=1024, QB=1024):
    """Emit the per-core attention program into `nc`. Requires S == 2*D."""
    assert S == 2 * D and S % P == 0 and D % P == 0 and QB % P == 0
    NST = S // P        # seq tiles (16)
    NDT = D // P        # d_model tiles (8)
    NQT = QB // P       # query tiles for this core (8)
    NC = min(512, D, QB)  # matmul free-dim chunk (one PSUM bank of f32)
    NCH_D = D // NC     # chunks over output channels (2)
    NCH_Q = QB // NC    # chunks over queries (2)
    EXP = mybir.ActivationFunctionType.Exp

    x_ap = nc.dram_tensor("x", [S, D], F32, kind="ExternalInput").ap()
    xq_ap = nc.dram_tensor("xq", [QB, D], F32, kind="ExternalInput").ap()
    w_aps = {
        w: nc.dram_tensor(w, [D, D], F32, kind="ExternalInput").ap()
        for w in ("wq", "wk", "wv")
    }
    out_ap = nc.dram_tensor("out", [QB, D], F32, kind="ExternalOutput").ap()

    CAST_ROWS = min(512, D)  # rows per f32->bf16 cast DMA chunk
    TR_ROWS = min(512, D)    # rows per xbar-transpose DMA chunk

    with tile.TileContext(nc) as tc, ExitStack() as ctx:
        const_pool = ctx.enter_context(tc.tile_pool(name="const", bufs=1))
        qt_pool = ctx.enter_context(tc.tile_pool(name="qt", bufs=1))
        kr_pool = ctx.enter_context(tc.tile_pool(name="kr", bufs=1))
        v_pool = ctx.enter_context(tc.tile_pool(name="v", bufs=1))
        dram = ctx.enter_context(tc.tile_pool(name="dram", bufs=1, space="DRAM"))
        psum_mm = ctx.enter_context(tc.tile_pool(name="psum_mm", bufs=6, space="PSUM"))

        ones = const_pool.tile([P, 1], BF16)
        nc.vector.memset(ones, 1.0)

        QT = [qt_pool.tile([P, QB], BF16, tag=f"QT{m}", name=f"QT{m}") for m in range(NDT)]
        KR = [kr_pool.tile([P, S], BF16, tag=f"KR{m}", name=f"KR{m}") for m in range(NDT)]
        V = [v_pool.tile([P, D], BF16, tag=f"V{s}", name=f"V{s}") for s in range(NST)]

        identity = const_pool.tile([P, P], F32)
        make_identity(nc, identity)

        with tc.tile_pool(name="xt", bufs=1) as xt_pool, \
                tc.tile_pool(name="wt", bufs=2) as wt_pool, \
                tc.tile_pool(name="stage", bufs=4) as stage, \
                tc.tile_pool(name="psum_t", bufs=2, space="PSUM") as psum_t:

            def cast_w_bf16(src_ap, wname):
                # W f32 DRAM -> bf16 DRAM scratch (SWDGE cast DMAs)
                dst = dram.tile([D, D], BF16, name=f"bf_{wname}")
                for r0 in range(0, D, CAST_ROWS):
                    nc.gpsimd.dma_start(
                        out=dst[r0:r0 + CAST_ROWS, :], in_=src_ap[r0:r0 + CAST_ROWS, :]
                    )
                return dst

            def load_transposed_w(w_bf, dstT):
                # bf16 DRAM [D, D] -> dstT: NDT bf16 SBUF tiles [P, D] (xbar)
                for dt in range(NDT):
                    for r0 in range(0, D, TR_ROWS):
                        nc.sync.dma_start(
                            out=dstT[dt][:, r0:r0 + TR_ROWS],
                            in_=w_bf[r0:r0 + TR_ROWS, dt * P:(dt + 1) * P],
                            transpose=True,
                        )

            def load_transposed_pe(src_ap, nrt, dstT):
                # f32 DRAM [nrt*P, D] -> dstT via PE transposes + DVE cast
                for rt in range(nrt):
                    nat = stage.tile([P, D], F32, tag="stage", name="nat")
                    nc.sync.dma_start(out=nat[:], in_=src_ap[rt * P:(rt + 1) * P, :])
                    for dt in range(NDT):
                        pt = psum_t.tile([P, P], F32, tag="pt", name="pt")
                        nc.tensor.transpose(pt[:], nat[:, dt * P:(dt + 1) * P], identity)
                        nc.vector.tensor_copy(dstT[dt][:, rt * P:(rt + 1) * P], pt[:])

            # W_Q cast kicked off first so the QT matmuls can start early
            w_bf = {w: cast_w_bf16(w_aps[w], w) for w in ("wq", "wk", "wv")}

            xqT = [xt_pool.tile([P, QB], BF16, tag=f"xqT{d}", name=f"xqT{d}") for d in range(NDT)]
            load_transposed_pe(xq_ap, NQT, xqT)
            xT = [xt_pool.tile([P, S], BF16, tag=f"xT{d}", name=f"xT{d}") for d in range(NDT)]
            load_transposed_pe(x_ap, NST, xT)
            # stride-2 views of xT: [P, 2, S//2]; [:, half, m] = xT[:, 2m+half]
            xTe = [t.rearrange("p (m two) -> p two m", two=2) for t in xT]

            for wname in ("wq", "wk", "wv"):
                wT = [wt_pool.tile([P, D], BF16, tag=f"wT{d}", name=f"wT{d}") for d in range(NDT)]
                load_transposed_w(w_bf[wname], wT)

                if wname == "wq":
                    # QT[mt][:, ich] = sum_dt wqT[dt][:, m-slice].T @ xqT[dt][:, ich]
                    for mt in range(NDT):
                        for ich in range(NCH_Q):
                            pm = psum_mm.tile([P, NC], F32, tag="pm")
                            for dt in range(NDT):
                                nc.tensor.matmul(
                                    pm[:],
                                    wT[dt][:, mt * P:(mt + 1) * P],
                                    xqT[dt][:, ich * NC:(ich + 1) * NC],
                                    start=(dt == 0), stop=(dt == NDT - 1),
                                )
                            nc.scalar.copy(QT[mt][:, ich * NC:(ich + 1) * NC], pm[:])
                elif wname == "wk":
                    # KR[mt][:, half*D + jch] = sum_dt xT[dt][:, 2m+half].T @ wkT[dt][:, jch]
                    for half in range(2):
                        for mt in range(NDT):
                            for jch in range(NCH_D):
                                pm = psum_mm.tile([P, NC], F32, tag="pm")
                                for dt in range(NDT):
                                    nc.tensor.matmul(
                                        pm[:],
                                        xTe[dt][:, half, mt * P:(mt + 1) * P],
                                        wT[dt][:, jch * NC:(jch + 1) * NC],
                                        start=(dt == 0), stop=(dt == NDT - 1),
                                    )
                                nc.scalar.copy(
                                    KR[mt][:, half * D + jch * NC: half * D + (jch + 1) * NC],
                                    pm[:],
                                )
                else:
                    # V[st][:, cch] = sum_dt xT[dt][:, s-slice].T @ wvT[dt][:, cch]
                    for st in range(NST):
                        for cch in range(NCH_D):
                            pm = psum_mm.tile([P, NC], F32, tag="pm")
                            for dt in range(NDT):
                                nc.tensor.matmul(
                                    pm[:],
                                    xT[dt][:, st * P:(st + 1) * P],
                                    wT[dt][:, cch * NC:(cch + 1) * NC],
                                    start=(dt == 0), stop=(dt == NDT - 1),
                                )
                            nc.vector.tensor_copy(V[st][:, cch * NC:(cch + 1) * NC], pm[:])

        with tc.tile_pool(name="et", bufs=1) as et_pool, \
                tc.tile_pool(name="ostage", bufs=3) as ostage, \
                tc.tile_pool(name="recip", bufs=1) as recip_pool, \
                tc.tile_pool(name="psum_r", bufs=2, space="PSUM") as psum_r:

            # scores^T and exp: ET[jt][:, ich] = exp(sum_mt KR.T @ QT / D)
            ET = [et_pool.tile([P, QB], BF16, tag=f"ET{j}", name=f"ET{j}") for j in range(NST)]
            for jt in range(NST):
                for ich in range(NCH_Q):
                    pm = psum_mm.tile([P, NC], F32, tag="pm")
                    for mt in range(NDT):
                        nc.tensor.matmul(
                            pm[:],
                            KR[mt][:, jt * P:(jt + 1) * P],
                            QT[mt][:, ich * NC:(ich + 1) * NC],
                            start=(mt == 0), stop=(mt == NDT - 1),
                        )
                    nc.scalar.activation(
                        ET[jt][:, ich * NC:(ich + 1) * NC], pm[:], EXP, scale=1.0 / D
                    )

            # softmax denominators, directly in [query-partition, 1] layout
            recips = []
            for it in range(NQT):
                pr = psum_r.tile([P, 1], F32, tag="pr")
                for jt in range(NST):
                    nc.tensor.matmul(
                        pr[:], ET[jt][:, it * P:(it + 1) * P], ones[:],
                        start=(jt == 0), stop=(jt == NST - 1),
                    )
                rc = recip_pool.tile([P, 1], F32, tag=f"rc{it}", name=f"rc{it}")
                nc.vector.reciprocal(rc[:], pr[:])
                recips.append(rc)

            # out[it][:, cch] = (sum_jt ET.T @ V) * recip[it]
            for it in range(NQT):
                for cch in range(NCH_D):
                    pm = psum_mm.tile([P, NC], F32, tag="pm")
                    for jt in range(NST):
                        nc.tensor.matmul(
                            pm[:],
                            ET[jt][:, it * P:(it + 1) * P],
                            V[jt][:, cch * NC:(cch + 1) * NC],
                            start=(jt == 0), stop=(jt == NST - 1),
                        )
                    ob = ostage.tile([P, NC], F32, tag="ob")
                    nc.vector.tensor_scalar_mul(ob[:], pm[:], recips[it][:])
                    nc.sync.dma_start(
                        out=out_ap[it * P:(it + 1) * P, cch * NC:(cch + 1) * NC],
                        in_=ob[:],
                    )
    return nc


_CACHE = {}


def _get_nc(S=2048, D=1024, QB=1024):
    key = (S, D, QB)
    if key not in _CACHE:
        nc = bacc.Bacc("TRN2", target_bir_lowering=False, debug=False, num_devices=8)
        build_attention(nc, S=S, D=D, QB=QB)
        nc.compile()
        _CACHE[key] = nc
    return _CACHE[key]


def _run(x, W_Q, W_K, W_V, **spmd_kwargs):
    B, S, D = x.shape  # (4, 2048, 1024)
    QB = S * B // 8    # queries per core (1024)
    x = np.ascontiguousarray(np.asarray(x, dtype=np.float32))
    ws = {
        "wq": np.ascontiguousarray(np.asarray(W_Q, dtype=np.float32)),
        "wk": np.ascontiguousarray(np.asarray(W_K, dtype=np.float32)),
        "wv": np.ascontiguousarray(np.asarray(W_V, dtype=np.float32)),
    }
    nc = _get_nc(S=S, D=D, QB=QB)
    in_maps = []
    for core in range(8):
        b, h = core // 2, core % 2
        in_maps.append({
            "x": x[b],
            "xq": np.ascontiguousarray(x[b, h * QB:(h + 1) * QB, :]),
            **ws,
        })
    res = run_bass_kernel_spmd(nc, in_maps, list(range(8)), **spmd_kwargs)
    out = np.empty((B, S, D), dtype=np.float32)
    for core in range(8):
        b, h = core // 2, core % 2
        out[b, h * QB:(h + 1) * QB, :] = res.results[core]["out"]
    return out, res


def kernel(x, W_Q, W_K, W_V):
    return _run(x, W_Q, W_K, W_V)[0]


# revision 11
# speedup vs baseline: 1.2355x; 1.1736x over previous
"""Trainium2 Bass kernel for single-head attention with row-major K-reshape.

Reference computation (per batch b):
    Q = x @ W_Q.T ; K = x @ W_K.T ; V = x @ W_V.T          # [S, D]
    K_r = K.reshape(D, S)          # row-major reshape, NOT a transpose
    scores = Q @ K_r / D
    out = softmax(scores, -1) @ V

Shapes: B=4, S=2048, D=1024, f32.

Sharding: 8 cores = (batch b in 0..3) x (query half h in 0..1). Each core
computes out[b, h*1024:(h+1)*1024, :] from full x[b] (K/V need the whole
sequence) plus its query block. No collectives.

Key layout identity: with S == 2*D the row-major reshape gives
    K_r[m, j]      = K[2m, j]        for j <  D
    K_r[m, D + j]  = K[2m+1, j]      for j <  D
so K_r is produced DIRECTLY by the K projection using stride-2 column
slices of x^T as the stationary operand:
    K_r[m, j]     = sum_d x^T[d, 2m]   * W_K^T[d, j]
    K_r[m, D + j] = sum_d x^T[d, 2m+1] * W_K^T[d, j]

Dataflow per core (TensorE matmul computes out[M,N] = lhsT[K,M].T @ rhs[K,N],
contraction over the partition dim):
    xT[d, s], xqT[d, i]: PE 128x128 transposes of the f32 natural tiles,
        bf16 cast on the PSUM->SBUF copy (DVE).  w*T[d, c]: f32->bf16 cast
        DMAs (SWDGE, DRAM->DRAM scratch) + xbar transpose-DMA loads.
    QT[m, i]   = lhsT=wqT[:, m-slice],        rhs=xqT            (proj)
    KR[m, j]   = lhsT=xT[:, stride-2 slice],  rhs=wkT            (proj)
    V[s, c]    = lhsT=xT[:, s-slice],         rhs=wvT            (proj)
    ST[j, i]   = lhsT=KR[:, j-slice],         rhs=QT             (scores^T)
    ET[j, i]   = exp(ST / D)                  (ACT, psum->sbuf bf16)
    rsum[i, 1] = lhsT=ET[:, i-slice],         rhs=ones[128, 1]   (row sums)
    O[i, c]    = lhsT=ET[:, i-slice],         rhs=V              (out)
    out        = O * (1 / rsum)               (DVE per-partition scalar)

All matmul operands bf16 (1 cycle/row on PE), accumulation f32 in PSUM.
"""

from contextlib import ExitStack

import numpy as np

import concourse.bass as bass
import concourse.tile as tile
from concourse import bacc, mybir
from concourse.bass_utils import run_bass_kernel_spmd
from concourse.masks import make_identity

F32 = mybir.dt.float32
BF16 = mybir.dt.bfloat16
P = 128


def build_attention(nc, S=2048, D=1024, QB=1024):
    """Emit the per-core attention program into `nc`. Requires S == 2*D."""
    assert S == 2 * D and S % P == 0 and D % P == 0 and QB % P == 0
    NST = S // P        # seq tiles (16)
    NDT = D // P        # d_model tiles (8)
    NQT = QB // P       # query tiles for this core (8)
    NC = min(512, D, QB)  # matmul free-dim chunk (one PSUM bank of f32)
    NCH_D = D // NC     # chunks over output channels (2)
    NCH_Q = QB // NC    # chunks over queries (2)
    EXP = mybir.ActivationFunctionType.Exp

    x_ap = nc.dram_tensor("x", [S, D], F32, kind="ExternalInput").ap()
    xq_ap = nc.dram_tensor("xq", [QB, D], F32, kind="ExternalInput").ap()
    w_aps = {
        w: nc.dram_tensor(w, [D, D], F32, kind="ExternalInput").ap()
        for w in ("wq", "wk", "wv")
    }
    out_ap = nc.dram_tensor("out", [QB, D], F32, kind="ExternalOutput").ap()

    CAST_ROWS = min(512, D)  # rows per f32->bf16 cast DMA chunk
    TR_ROWS = min(512, D)    # rows per xbar-transpose DMA chunk

    with tile.TileContext(nc) as tc, ExitStack() as ctx:
        const_pool = ctx.enter_context(tc.tile_pool(name="const", bufs=1))
        qt_pool = ctx.enter_context(tc.tile_pool(name="qt", bufs=1))
        kr_pool = ctx.enter_context(tc.tile_pool(name="kr", bufs=1))
        v_pool = ctx.enter_context(tc.tile_pool(name="v", bufs=1))
        dram = ctx.enter_context(tc.tile_pool(name="dram", bufs=1, space="DRAM"))
        psum_mm = ctx.enter_context(tc.tile_pool(name="psum_mm", bufs=4, space="PSUM"))

        ones = const_pool.tile([P, 1], BF16)
        nc.vector.memset(ones, 1.0)

        QT = [qt_pool.tile([P, QB], BF16, tag=f"QT{m}", name=f"QT{m}") for m in range(NDT)]
        KR = [kr_pool.tile([P, S], BF16, tag=f"KR{m}", name=f"KR{m}") for m in range(NDT)]
        V = [v_pool.tile([P, D], BF16, tag=f"V{s}", name=f"V{s}") for s in range(NST)]

        identity = const_pool.tile([P, P], F32)
        make_identity(nc, identity)

        with tc.tile_pool(name="xt", bufs=1) as xt_pool, \
                tc.tile_pool(name="wt", bufs=2) as wt_pool, \
                tc.tile_pool(name="stage", bufs=6) as stage, \
                tc.tile_pool(name="psum_t", bufs=4, space="PSUM") as psum_t:

            def cast_w_bf16(src_ap, wname):
                # W f32 DRAM -> bf16 DRAM scratch (one SWDGE cast DMA)
                dst = dram.tile([D, D], BF16, name=f"bf_{wname}")
                nc.gpsimd.dma_start(out=dst[:, :], in_=src_ap[:, :])
                return dst

            def load_transposed_w(w_bf, dstT):
                # bf16 DRAM [D, D] -> dstT: NDT bf16 SBUF tiles [P, D] (xbar)
                for dt in range(NDT):
                    nc.sync.dma_start(
                        out=dstT[dt][:, :],
                        in_=w_bf[:, dt * P:(dt + 1) * P],
                        transpose=True,
                    )

            def load_transposed_pe(src_ap, nrt, dstT):
                # f32 DRAM [nrt*P, D] -> dstT via PE transposes + DVE cast
                for rt in range(nrt):
                    nat = stage.tile([P, D], F32, tag="stage", name="nat")
                    nc.sync.dma_start(out=nat[:], in_=src_ap[rt * P:(rt + 1) * P, :])
                    for dt in range(NDT):
                        pt = psum_t.tile([P, P], F32, tag="pt", name="pt")
                        nc.tensor.transpose(pt[:], nat[:, dt * P:(dt + 1) * P], identity)
                        nc.vector.tensor_copy(dstT[dt][:, rt * P:(rt + 1) * P], pt[:])

            # W_Q cast kicked off first so the QT matmuls can start early
            w_bf = {w: cast_w_bf16(w_aps[w], w) for w in ("wq", "wk", "wv")}

            xqT = [xt_pool.tile([P, QB], BF16, tag=f"xqT{d}", name=f"xqT{d}") for d in range(NDT)]
            load_transposed_pe(xq_ap, NQT, xqT)
            xT = [xt_pool.tile([P, S], BF16, tag=f"xT{d}", name=f"xT{d}") for d in range(NDT)]
            # stride-2 views of xT: [P, 2, S//2]; [:, half, m] = xT[:, 2m+half]
            xTe = [t.rearrange("p (m two) -> p two m", two=2) for t in xT]

            for wname in ("wq", "wk", "wv"):
                if wname == "wk":
                    # emitted after QT so those matmuls overlap the x loads
                    load_transposed_pe(x_ap, NST, xT)
                wT = [wt_pool.tile([P, D], BF16, tag=f"wT{d}", name=f"wT{d}") for d in range(NDT)]
                load_transposed_w(w_bf[wname], wT)

                if wname == "wq":
                    # QT[mt][:, ich] = sum_dt wqT[dt][:, m-slice].T @ xqT[dt][:, ich]
                    for mt in range(NDT):
                        for ich in range(NCH_Q):
                            pm = psum_mm.tile([P, NC], F32, tag="pm")
                            for dt in range(NDT):
                                nc.tensor.matmul(
                                    pm[:],
                                    wT[dt][:, mt * P:(mt + 1) * P],
                                    xqT[dt][:, ich * NC:(ich + 1) * NC],
                                    start=(dt == 0), stop=(dt == NDT - 1),
                                )
                            nc.scalar.copy(QT[mt][:, ich * NC:(ich + 1) * NC], pm[:])
                elif wname == "wk":
                    # KR[mt][:, half*D + jch] = sum_dt xT[dt][:, 2m+half].T @ wkT[dt][:, jch]
                    for half in range(2):
                        for mt in range(NDT):
                            for jch in range(NCH_D):
                                pm = psum_mm.tile([P, NC], F32, tag="pm")
                                for dt in range(NDT):
                                    nc.tensor.matmul(
                                        pm[:],
                                        xTe[dt][:, half, mt * P:(mt + 1) * P],
                                        wT[dt][:, jch * NC:(jch + 1) * NC],
                                        start=(dt == 0), stop=(dt == NDT - 1),
                                    )
                                nc.scalar.copy(
                                    KR[mt][:, half * D + jch * NC: half * D + (jch + 1) * NC],
                                    pm[:],
                                )
                else:
                    # V[st][:, cch] = sum_dt xT[dt][:, s-slice].T @ wvT[dt][:, cch]
                    for st in range(NST):
                        for cch in range(NCH_D):
                            pm = psum_mm.tile([P, NC], F32, tag="pm")
                            for dt in range(NDT):
                                nc.tensor.matmul(
                                    pm[:],
                                    xT[dt][:, st * P:(st + 1) * P],
                                    wT[dt][:, cch * NC:(cch + 1) * NC],
                                    start=(dt == 0), stop=(dt == NDT - 1),
                                )
                            nc.vector.tensor_copy(V[st][:, cch * NC:(cch + 1) * NC], pm[:])

        with tc.tile_pool(name="et", bufs=1) as et_pool, \
                tc.tile_pool(name="ostage", bufs=3) as ostage, \
                tc.tile_pool(name="recip", bufs=1) as recip_pool, \
                tc.tile_pool(name="psum_r", bufs=2, space="PSUM") as psum_r:

            # scores^T and exp: ET[jt][:, ich] = exp(sum_mt KR.T @ QT / D)
            ET = [et_pool.tile([P, QB], BF16, tag=f"ET{j}", name=f"ET{j}") for j in range(NST)]
            for jt in range(NST):
                for ich in range(NCH_Q):
                    pm = psum_mm.tile([P, NC], F32, tag="pm")
                    for mt in range(NDT):
                        nc.tensor.matmul(
                            pm[:],
                            KR[mt][:, jt * P:(jt + 1) * P],
                            QT[mt][:, ich * NC:(ich + 1) * NC],
                            start=(mt == 0), stop=(mt == NDT - 1),
                        )
                    nc.scalar.activation(
                        ET[jt][:, ich * NC:(ich + 1) * NC], pm[:], EXP, scale=1.0 / D
                    )

            # softmax denominators, directly in [query-partition, 1] layout
            recips = []
            for it in range(NQT):
                pr = psum_r.tile([P, 1], F32, tag="pr")
                for jt in range(NST):
                    nc.tensor.matmul(
                        pr[:], ET[jt][:, it * P:(it + 1) * P], ones[:],
                        start=(jt == 0), stop=(jt == NST - 1),
                    )
                rc = recip_pool.tile([P, 1], F32, tag=f"rc{it}", name=f"rc{it}")
                nc.vector.reciprocal(rc[:], pr[:])
                recips.append(rc)

            # out[it][:, cch] = (sum_jt ET.T @ V) * recip[it]
            for it in range(NQT):
                for cch in range(NCH_D):
                    pm = psum_mm.tile([P, NC], F32, tag="pm")
                    for jt in range(NST):
                        nc.tensor.matmul(
                            pm[:],
                            ET[jt][:, it * P:(it + 1) * P],
                            V[jt][:, cch * NC:(cch + 1) * NC],
                            start=(jt == 0), stop=(jt == NST - 1),
                        )
                    ob = ostage.tile([P, NC], F32, tag="ob")
                    nc.vector.tensor_scalar_mul(ob[:], pm[:], recips[it][:])
                    nc.sync.dma_start(
                        out=out_ap[it * P:(it + 1) * P, cch * NC:(cch + 1) * NC],
                        in_=ob[:],
                    )
    return nc


_CACHE = {}


def _get_nc(S=2048, D=1024, QB=1024):
    key = (S, D, QB)
    if key not in _CACHE:
        nc = bacc.Bacc("TRN2", target_bir_lowering=False, debug=False, num_devices=8)
        build_attention(nc, S=S, D=D, QB=QB)
        nc.compile()
        _CACHE[key] = nc
    return _CACHE[key]


def _run(x, W_Q, W_K, W_V, **spmd_kwargs):
    B, S, D = x.shape  # (4, 2048, 1024)
    QB = S * B // 8    # queries per core (1024)
    x = np.ascontiguousarray(np.asarray(x, dtype=np.float32))
    ws = {
        "wq": np.ascontiguousarray(np.asarray(W_Q, dtype=np.float32)),
        "wk": np.ascontiguousarray(np.asarray(W_K, dtype=np.float32)),
        "wv": np.ascontiguousarray(np.asarray(W_V, dtype=np.float32)),
    }
    nc = _get_nc(S=S, D=D, QB=QB)
    in_maps = []
    for core in range(8):
        b, h = core // 2, core % 2
        in_maps.append({
            "x": x[b],
            "xq": np.ascontiguousarray(x[b, h * QB:(h + 1) * QB, :]),
            **ws,
        })
    res = run_bass_kernel_spmd(nc, in_maps, list(range(8)), **spmd_kwargs)
    out = np.empty((B, S, D), dtype=np.float32)
    for core in range(8):
        b, h = core // 2, core % 2
        out[b, h * QB:(h + 1) * QB, :] = res.results[core]["out"]
    return out, res


def kernel(x, W_Q, W_K, W_V):
    return _run(x, W_Q, W_K, W_V)[0]


# revision 12
# speedup vs baseline: 1.3633x; 1.1034x over previous
"""Trainium2 Bass kernel for single-head attention with row-major K-reshape.

Reference computation (per batch b):
    Q = x @ W_Q.T ; K = x @ W_K.T ; V = x @ W_V.T          # [S, D]
    K_r = K.reshape(D, S)          # row-major reshape, NOT a transpose
    scores = Q @ K_r / D
    out = softmax(scores, -1) @ V

Shapes: B=4, S=2048, D=1024, f32.

Sharding: 8 cores = (batch b in 0..3) x (query half h in 0..1). Each core
computes out[b, h*1024:(h+1)*1024, :] from full x[b] (K/V need the whole
sequence) plus its query block. No collectives.

Key layout identity: with S == 2*D the row-major reshape gives
    K_r[m, j]      = K[2m, j]        for j <  D
    K_r[m, D + j]  = K[2m+1, j]      for j <  D
so K_r is produced DIRECTLY by the K projection using stride-2 column
slices of x^T as the stationary operand:
    K_r[m, j]     = sum_d x^T[d, 2m]   * W_K^T[d, j]
    K_r[m, D + j] = sum_d x^T[d, 2m+1] * W_K^T[d, j]

Dataflow per core (TensorE matmul computes out[M,N] = lhsT[K,M].T @ rhs[K,N],
contraction over the partition dim):
    xT[d, s], xqT[d, i]: PE 128x128 transposes of the f32 natural tiles,
        bf16 cast on the PSUM->SBUF copy (DVE).  w*T[d, c]: f32->bf16 cast
        DMAs (SWDGE, DRAM->DRAM scratch) + xbar transpose-DMA loads.
    QT[m, i]   = lhsT=wqT[:, m-slice],        rhs=xqT            (proj)
    KR[m, j]   = lhsT=xT[:, stride-2 slice],  rhs=wkT            (proj)
    V[s, c]    = lhsT=xT[:, s-slice],         rhs=wvT            (proj)
    ST[j, i]   = lhsT=KR[:, j-slice],         rhs=QT             (scores^T)
    ET[j, i]   = exp(ST / D)                  (ACT, psum->sbuf bf16)
    rsum[i, 1] = lhsT=ET[:, i-slice],         rhs=ones[128, 1]   (row sums)
    O[i, c]    = lhsT=ET[:, i-slice],         rhs=V              (out)
    out        = O * (1 / rsum)               (DVE per-partition scalar)

All matmul operands bf16 (1 cycle/row on PE), accumulation f32 in PSUM.
"""

from contextlib import ExitStack

import numpy as np

import concourse.bass as bass
import concourse.tile as tile
from concourse import bacc, mybir
from concourse.bass_utils import run_bass_kernel_spmd
from concourse.masks import make_identity

F32 = mybir.dt.float32
BF16 = mybir.dt.bfloat16
P = 128


def build_attention(nc, S=2048, D=1024, QB=1024):
    """Emit the per-core attention program into `nc`. Requires S == 2*D."""
    assert S == 2 * D and S % P == 0 and D % P == 0 and QB % P == 0
    NST = S // P        # seq tiles (16)
    NDT = D // P        # d_model tiles (8)
    NQT = QB // P       # query tiles for this core (8)
    NC = min(512, D, QB)  # matmul free-dim chunk (one PSUM bank of f32)
    NCH_D = D // NC     # chunks over output channels (2)
    NCH_Q = QB // NC    # chunks over queries (2)
    EXP = mybir.ActivationFunctionType.Exp

    x_ap = nc.dram_tensor("x", [S, D], F32, kind="ExternalInput").ap()
    xq_ap = nc.dram_tensor("xq", [QB, D], F32, kind="ExternalInput").ap()
    w_aps = {
        w: nc.dram_tensor(w, [D, D], F32, kind="ExternalInput").ap()
        for w in ("wq", "wk", "wv")
    }
    out_ap = nc.dram_tensor("out", [QB, D], F32, kind="ExternalOutput").ap()

    CAST_ROWS = min(512, D)  # rows per f32->bf16 cast DMA chunk
    TR_ROWS = min(512, D)    # rows per xbar-transpose DMA chunk

    with tile.TileContext(nc) as tc, ExitStack() as ctx:
        const_pool = ctx.enter_context(tc.tile_pool(name="const", bufs=1))
        qt_pool = ctx.enter_context(tc.tile_pool(name="qt", bufs=1))
        kr_pool = ctx.enter_context(tc.tile_pool(name="kr", bufs=1))
        v_pool = ctx.enter_context(tc.tile_pool(name="v", bufs=1))
        dram = ctx.enter_context(tc.tile_pool(name="dram", bufs=1, space="DRAM"))
        psum_mm = ctx.enter_context(tc.tile_pool(name="psum_mm", bufs=4, space="PSUM"))

        ones = const_pool.tile([P, 1], BF16)
        nc.vector.memset(ones, 1.0)

        QT = [qt_pool.tile([P, QB], BF16, tag=f"QT{m}", name=f"QT{m}") for m in range(NDT)]
        KR = [kr_pool.tile([P, S], BF16, tag=f"KR{m}", name=f"KR{m}") for m in range(NDT)]
        V = [v_pool.tile([P, D], BF16, tag=f"V{s}", name=f"V{s}") for s in range(NST)]

        identity = const_pool.tile([P, P], F32)
        make_identity(nc, identity)

        with tc.tile_pool(name="xt", bufs=1) as xt_pool, \
                tc.tile_pool(name="wt", bufs=2) as wt_pool, \
                tc.tile_pool(name="stage", bufs=6) as stage, \
                tc.tile_pool(name="psum_t", bufs=4, space="PSUM") as psum_t:

            def cast_w_bf16(src_ap, wname):
                # W f32 DRAM -> bf16 DRAM scratch (one SWDGE cast DMA)
                dst = dram.tile([D, D], BF16, name=f"bf_{wname}")
                nc.gpsimd.dma_start(out=dst[:, :], in_=src_ap[:, :])
                return dst

            def load_transposed_w(w_bf, dstT):
                # bf16 DRAM [D, D] -> dstT: NDT bf16 SBUF tiles [P, D] (xbar)
                for dt in range(NDT):
                    nc.sync.dma_start(
                        out=dstT[dt][:, :],
                        in_=w_bf[:, dt * P:(dt + 1) * P],
                        transpose=True,
                    )

            def load_transposed_pe(src_ap, nrt, dstT):
                # f32 DRAM [nrt*P, D] -> dstT via PE transposes + DVE cast
                for rt in range(nrt):
                    nat = stage.tile([P, D], F32, tag="stage", name="nat")
                    nc.sync.dma_start(out=nat[:], in_=src_ap[rt * P:(rt + 1) * P, :])
                    for dt in range(NDT):
                        pt = psum_t.tile([P, P], F32, tag="pt", name="pt")
                        nc.tensor.transpose(pt[:], nat[:, dt * P:(dt + 1) * P], identity)
                        nc.vector.tensor_copy(dstT[dt][:, rt * P:(rt + 1) * P], pt[:])

            # W_K/W_V cast via SWDGE->xbar; W_Q goes through the PE-transpose
            # path below so the QT matmuls can start as early as possible.
            w_bf = {w: cast_w_bf16(w_aps[w], w) for w in ("wk", "wv")}

            xqT = [xt_pool.tile([P, QB], BF16, tag=f"xqT{d}", name=f"xqT{d}") for d in range(NDT)]
            load_transposed_pe(xq_ap, NQT, xqT)
            xT = [xt_pool.tile([P, S], BF16, tag=f"xT{d}", name=f"xT{d}") for d in range(NDT)]
            # stride-2 views of xT: [P, 2, S//2]; [:, half, m] = xT[:, 2m+half]
            xTe = [t.rearrange("p (m two) -> p two m", two=2) for t in xT]

            for wname in ("wq", "wk", "wv"):
                wT = [wt_pool.tile([P, D], BF16, tag=f"wT{d}", name=f"wT{d}") for d in range(NDT)]
                if wname == "wq":
                    load_transposed_pe(w_aps["wq"], NDT, wT)
                else:
                    load_transposed_w(w_bf[wname], wT)

                if wname == "wq":
                    # QT[mt][:, ich] = sum_dt wqT[dt][:, m-slice].T @ xqT[dt][:, ich]
                    for mt in range(NDT):
                        for ich in range(NCH_Q):
                            pm = psum_mm.tile([P, NC], F32, tag="pm")
                            for dt in range(NDT):
                                nc.tensor.matmul(
                                    pm[:],
                                    wT[dt][:, mt * P:(mt + 1) * P],
                                    xqT[dt][:, ich * NC:(ich + 1) * NC],
                                    start=(dt == 0), stop=(dt == NDT - 1),
                                )
                            nc.scalar.copy(QT[mt][:, ich * NC:(ich + 1) * NC], pm[:])
                elif wname == "wk":
                    # KR[mt][:, half*D + jch] = sum_dt xT[dt][:, 2m+half].T @ wkT[dt][:, jch]
                    for mt in range(NDT):
                        # x rows 2mt*P .. (2mt+2)*P feed this mt block; transpose
                        # them here so KR matmuls interleave with x loads
                        load_transposed_pe(
                            x_ap[2 * mt * P:(2 * mt + 2) * P, :], 2,
                            [t[:, 2 * mt * P:(2 * mt + 2) * P] for t in xT],
                        )
                        for half in range(2):
                            for jch in range(NCH_D):
                                pm = psum_mm.tile([P, NC], F32, tag="pm")
                                for dt in range(NDT):
                                    nc.tensor.matmul(
                                        pm[:],
                                        xTe[dt][:, half, mt * P:(mt + 1) * P],
                                        wT[dt][:, jch * NC:(jch + 1) * NC],
                                        start=(dt == 0), stop=(dt == NDT - 1),
                                    )
                                nc.scalar.copy(
                                    KR[mt][:, half * D + jch * NC: half * D + (jch + 1) * NC],
                                    pm[:],
                                )
                else:
                    # V[st][:, cch] = sum_dt xT[dt][:, s-slice].T @ wvT[dt][:, cch]
                    for st in range(NST):
                        for cch in range(NCH_D):
                            pm = psum_mm.tile([P, NC], F32, tag="pm")
                            for dt in range(NDT):
                                nc.tensor.matmul(
                                    pm[:],
                                    xT[dt][:, st * P:(st + 1) * P],
                                    wT[dt][:, cch * NC:(cch + 1) * NC],
                                    start=(dt == 0), stop=(dt == NDT - 1),
                                )
                            nc.vector.tensor_copy(V[st][:, cch * NC:(cch + 1) * NC], pm[:])

        with tc.tile_pool(name="et", bufs=1) as et_pool, \
                tc.tile_pool(name="ostage", bufs=3) as ostage, \
                tc.tile_pool(name="recip", bufs=1) as recip_pool, \
                tc.tile_pool(name="psum_r", bufs=2, space="PSUM") as psum_r:

            # scores^T and exp: ET[jt][:, ich] = exp(sum_mt KR.T @ QT / D)
            ET = [et_pool.tile([P, QB], BF16, tag=f"ET{j}", name=f"ET{j}") for j in range(NST)]
            for jt in range(NST):
                for ich in range(NCH_Q):
                    pm = psum_mm.tile([P, NC], F32, tag="pm")
                    for mt in range(NDT):
                        nc.tensor.matmul(
                            pm[:],
                            KR[mt][:, jt * P:(jt + 1) * P],
                            QT[mt][:, ich * NC:(ich + 1) * NC],
                            start=(mt == 0), stop=(mt == NDT - 1),
                        )
                    nc.scalar.activation(
                        ET[jt][:, ich * NC:(ich + 1) * NC], pm[:], EXP, scale=1.0 / D
                    )

            # softmax denominators, directly in [query-partition, 1] layout
            recips = []
            for it in range(NQT):
                pr = psum_r.tile([P, 1], F32, tag="pr")
                for jt in range(NST):
                    nc.tensor.matmul(
                        pr[:], ET[jt][:, it * P:(it + 1) * P], ones[:],
                        start=(jt == 0), stop=(jt == NST - 1),
                    )
                rc = recip_pool.tile([P, 1], F32, tag=f"rc{it}", name=f"rc{it}")
                nc.vector.reciprocal(rc[:], pr[:])
                recips.append(rc)

            # out[it][:, cch] = (sum_jt ET.T @ V) * recip[it]
            for it in range(NQT):
                for cch in range(NCH_D):
                    pm = psum_mm.tile([P, NC], F32, tag="pm")
                    for jt in range(NST):
                        nc.tensor.matmul(
                            pm[:],
                            ET[jt][:, it * P:(it + 1) * P],
                            V[jt][:, cch * NC:(cch + 1) * NC],
                            start=(jt == 0), stop=(jt == NST - 1),
                        )
                    ob = ostage.tile([P, NC], F32, tag="ob")
                    nc.vector.tensor_scalar_mul(ob[:], pm[:], recips[it][:])
                    nc.sync.dma_start(
                        out=out_ap[it * P:(it + 1) * P, cch * NC:(cch + 1) * NC],
                        in_=ob[:],
                    )
    return nc


_CACHE = {}


def _get_nc(S=2048, D=1024, QB=1024):
    key = (S, D, QB)
    if key not in _CACHE:
        nc = bacc.Bacc("TRN2", target_bir_lowering=False, debug=False, num_devices=8)
        build_attention(nc, S=S, D=D, QB=QB)
        nc.compile()
        _CACHE[key] = nc
    return _CACHE[key]


def _run(x, W_Q, W_K, W_V, **spmd_kwargs):
    B, S, D = x.shape  # (4, 2048, 1024)
    QB = S * B // 8    # queries per core (1024)
    x = np.ascontiguousarray(np.asarray(x, dtype=np.float32))
    ws = {
        "wq": np.ascontiguousarray(np.asarray(W_Q, dtype=np.float32)),
        "wk": np.ascontiguousarray(np.asarray(W_K, dtype=np.float32)),
        "wv": np.ascontiguousarray(np.asarray(W_V, dtype=np.float32)),
    }
    nc = _get_nc(S=S, D=D, QB=QB)
    in_maps = []
    for core in range(8):
        b, h = core // 2, core % 2
        in_maps.append({
            "x": x[b],
            "xq": np.ascontiguousarray(x[b, h * QB:(h + 1) * QB, :]),
            **ws,
        })
    res = run_bass_kernel_spmd(nc, in_maps, list(range(8)), **spmd_kwargs)
    out = np.empty((B, S, D), dtype=np.float32)
    for core in range(8):
        b, h = core // 2, core % 2
        out[b, h * QB:(h + 1) * QB, :] = res.results[core]["out"]
    return out, res


def kernel(x, W_Q, W_K, W_V):
    return _run(x, W_Q, W_K, W_V)[0]
